# revision 51
# baseline (speedup 1.0000x reference)
"""Trainium2 Bass kernel for nn_BinaryLinear (binarized 4-layer MLP + BatchNorm).

Reference computation (fp32, jax):
    h = x.reshape(-1, 3072)
    h = relu(h @ sign(W1).T); h = BN(h, g1, b1)   # BN over full 8192 batch
    h = relu(h @ sign(W2).T); h = BN(h, g2, b2)
    h = relu(h @ sign(W3).T); h = BN(h, g3, b3)
    out = h @ sign(W4).T                          # [8192, 10]

Strategy (8 NeuronCores, data-parallel over batch):
  - Host: binarize weights to fp8e4 (+-1 exact, halves the replicated-weight
    HBM feed), x transposed to bf16 [feature, batch] and packed partition-
    major ([128, ktiles, free]); shard x over cores (1024 rows each).
  - Device (SPMD): activations live feature-major [feat_part, batch_free].
    Each layer: K-tiled matmuls (fp8 stationary x bf16 moving, full rate)
    accumulating in PSUM. Feature tiles in stats groups [0-3],[4,5],[6,7]:
    the 4-wide group runs k-outer (consumes the previous layer's tiles /
    DMA'd XT tiles in BN-ready/arrival order), the rest are chains. The
    [6,7] chains run their two 512-batch halves k-major so the group's
    stats land right after j7's last matmul (sum via relu accum_out on
    ScalarE, sum-of-squares on VectorE from the bf16 relu copy).
  - BatchNorm over the full 8192 batch via one small AllGather per stats
    group (3/layer, staggered) + local tree-sum (AllGather beats AllReduce
    ~2-3x on latency for tiny payloads). cc_in kicks + result gathers ride
    the Sync ring (idle after the feed); collective triggers ride GpSimd.
    Stats math is ~8 tiny VectorE ops + one ScalarE sqrt (variance fused as
    (sq/B+eps)-mean^2 in one scalar_tensor_tensor); apply is one
    tensor_scalar per feature tile into ping-ponged H/H2 so the next layer's
    k-outer can consume tiles the moment they are normalized, deferring the
    last tiles (k=6,7) to the end of its k-loop. fin(g0) is issued after j5
    for L1 (applies must precede the next layer's k=0) but deferred past the
    j6/j7 stats for L2/L3 so a late AllGather cannot stall them on Vector.
  - All weights (fp8) + BNP are DMA'd up-front in growing chunks on the two
    HWDGE rings (Sync: XT+W2+BNP, Scalar: W1+W3+W4) — feed pacing uses
    small-to-4-tile chunks because larger tail chunks starve slow-HBM peer
    cores' k-outer and inflate every L1 AllGather wait. A warmup AllGather
    absorbs the ncfw collective-subsystem wake (~70us) off the critical path.
"""
import os
import sys

for _p in ("/opt/trn_rl_repo",):
    if os.path.isdir(_p) and _p not in sys.path:
        sys.path.insert(0, _p)

import numpy as np
import ml_dtypes

from concourse import bacc, tile, mybir
from concourse import bass_utils

NCORES = 8
B = 8192
BL = B // NCORES            # 1024 rows per core
KIN = 3072
KT_IN = KIN // 128          # 24 k-tiles for layer 1
HID = 1024
JT = HID // 128             # 8 feature tiles
CLS = 10
CLSP = 16                   # padded classes
EPS = 1e-5
BF16 = mybir.dt.bfloat16
FP8 = mybir.dt.float8e4
F32 = mybir.dt.float32
ADD = mybir.AluOpType.add
SUB = mybir.AluOpType.subtract
MUL = mybir.AluOpType.mult
MAX = mybir.AluOpType.max
BYP = mybir.AluOpType.bypass
RELU = mybir.ActivationFunctionType.Relu
IDENT = mybir.ActivationFunctionType.Identity

_CACHE = {}

GROUPS = [[0, 1, 2, 3], [4, 5], [6, 7]]
NHALVES = [(s, min(512, BL - s)) for s in range(0, BL, 512)]


def _build():
    nc = bacc.Bacc("TRN2", target_bir_lowering=False, debug=False, num_devices=NCORES)

    xt_d = nc.dram_tensor("xt", [128, KT_IN, BL], BF16, kind="ExternalInput")
    w1_d = nc.dram_tensor("w1t", [128, KT_IN, HID], FP8, kind="ExternalInput")
    w2_d = nc.dram_tensor("w2t", [128, JT, HID], FP8, kind="ExternalInput")
    w3_d = nc.dram_tensor("w3t", [128, JT, HID], FP8, kind="ExternalInput")
    w4_d = nc.dram_tensor("w4t", [128, JT, CLSP], FP8, kind="ExternalInput")
    bnp_d = nc.dram_tensor("bnp", [128, 6 * JT], F32, kind="ExternalInput")
    out_d = nc.dram_tensor("out", [CLSP, BL], F32, kind="ExternalOutput")

    with tile.TileContext(nc) as tc:
        with (
            tc.tile_pool(name="weights", bufs=1) as wpool,
            tc.tile_pool(name="acts", bufs=1) as apool,
            tc.tile_pool(name="scratch", bufs=2) as scrpool,
            tc.tile_pool(name="stats", bufs=3) as spool,
            tc.tile_pool(name="psum", bufs=4, space="PSUM") as pspool,
            tc.tile_pool(name="dram", bufs=3, space="DRAM") as dpool,
        ):
            XT = wpool.tile([128, KT_IN, BL], BF16, tag="XT")
            W1 = wpool.tile([128, KT_IN, HID], FP8, tag="W1")
            W2 = wpool.tile([128, JT, HID], FP8, tag="W2")
            W3 = wpool.tile([128, JT, HID], FP8, tag="W3")
            W4 = wpool.tile([128, JT, CLSP], FP8, tag="W4")
            BNP = wpool.tile([128, 6 * JT], F32, tag="BNP")
            HRAW = apool.tile([128, JT, BL], BF16, tag="HRAW")
            H = apool.tile([128, JT, BL], BF16, tag="H")
            H2 = apool.tile([128, JT, BL], BF16, tag="H2")


            # Warmup collective: absorbs the ncfw wake latency off the
            # critical path. Output anchored into an unused out_d row so DCE
            # keeps it.
            wu_in = dpool.tile([128, 1], F32, tag="wu_in")
            wu_out = dpool.tile([128, 1], F32, tag="wu_out")
            wu_gat = dpool.tile([NCORES * 128, 1], F32, tag="wu_gat")
            nc.gpsimd.collective_compute(
                "AllGather",
                BYP,
                replica_groups=[list(range(NCORES))],
                ins=[wu_in.opt()],
                outs=[wu_gat.opt()],
            )
            # (anchor DMA for wu_out is issued at the very end, on Sync — a
            # gpsimd anchor here would head-of-line block the cc_in kicks
            # behind the warmup collective's completion)

            # Input feed: XT on the Sync ring, W1 on the Scalar ring, in
            # progressively larger chunks so the first accumulation chains
            # start early. All remaining weights follow immediately (the
            # rings are idle afterwards and no bulk DMA remains mid-kernel).
            # first kicks cover exactly what mm[0] / the first LDWEIGHTS
            # need (XT tile-0 half-0, W1 tile-0 j-block-0) so the pipe
            # starts on a 128KB/16KB transfer instead of full tiles
            nc.sync.dma_start(XT[:, 0:1, 0:512], xt_d[:, 0:1, 0:512])
            nc.scalar.dma_start(W1[:, 0:1, 0:128], w1_d[:, 0:1, 0:128])
            nc.sync.dma_start(XT[:, 0:1, 512:BL], xt_d[:, 0:1, 512:BL])
            nc.scalar.dma_start(W1[:, 0:1, 128:HID], w1_d[:, 0:1, 128:HID])
            nc.sync.dma_start(BNP[:], bnp_d[:])
            nc.scalar.dma_start(W4[:], w4_d[:])
            feed = [1, 2, 4, 4, 4, 4, 4]
            c = 1
            for w in feed:
                w = min(w, KT_IN - c)
                if w <= 0:
                    break
                nc.sync.dma_start(XT[:, c : c + w, :], xt_d[:, c : c + w, :])
                nc.scalar.dma_start(W1[:, c : c + w, :], w1_d[:, c : c + w, :])
                c += w
            nc.sync.dma_start(W2[:], w2_d[:])
            nc.scalar.dma_start(W3[:], w3_d[:])

            def mm_pair(ps, Wk, rhs, k, kt):
                for idx, (s, w) in enumerate(NHALVES):
                    mi = nc.tensor.matmul(
                        ps[:, s : s + w],
                        Wk,
                        rhs[:, k, s : s + w],
                        start=(k == 0),
                        stop=(k == kt - 1),
                    )
                    if idx > 0:
                        # same stationary weights as the previous matmul:
                        # skip the redundant LDWEIGHTS
                        mi.ins.ldweights = False

            def relu_sumsq(ps, jt, S, i, n):
                # relu: PSUM f32 -> SBUF bf16; accum = per-feature batch sum
                nc.scalar.activation(
                    HRAW[:, jt, :], ps[:], RELU, accum_out=S[:, i : i + 1]
                )
                # sum of relu^2 over batch (VectorE: h*h from the bf16 copy)
                scr = scrpool.tile([128, BL], BF16, tag="scr", name=f"scr_{jt}")
                nc.vector.scalar_tensor_tensor(
                    scr[:], HRAW[:, jt, :], 0.0, HRAW[:, jt, :], BYP, MUL,
                    accum_out=S[:, n + i : n + i + 1],
                )

            def ar_start(li, g, S):
                """Kick stats to DRAM + trigger the AllGather (GpSimd queue).

                AllGather + local tree-sum beats AllReduce here: tiny-payload
                AllReduce is two serialization phases (reduce-scatter +
                gather), each paying the cross-core skew — measured 25-30us
                end-to-end vs ~7-13us for AllGather."""
                m = S.shape[1]
                cc_in = dpool.tile([128, m], F32, tag="cc_in",
                                   name=f"cci_{li}_{g}")
                cc_out = dpool.tile([NCORES * 128, m], F32, tag="cc_out",
                                    name=f"cco_{li}_{g}")
                nc.sync.dma_start(cc_in[:], S[:])
                nc.gpsimd.collective_compute(
                    "AllGather",
                    BYP,
                    replica_groups=[list(range(NCORES))],
                    ins=[cc_in.opt()],
                    outs=[cc_out.opt()],
                )
                return cc_out

            def bn_finish(li, g, cc_out, Hdst, split=False):
                """Gather AllReduce result, stats math, apply into Hdst."""
                jts = GROUPS[g]
                n = len(jts)
                m = cc_out.shape[1]
                nm = f"{li}_{g}"
                # gather the 8 cores' contributions side by side (contiguous
                # per-core blocks — a cores-innermost layout makes the DMA
                # 4-byte-strided and ~10us slow), then tree-sum
                R = spool.tile([128, NCORES, m], F32, tag="R", name=f"R_{nm}")
                nc.sync.dma_start(
                    R[:], cc_out.opt().rearrange("(c p) s -> p c s", p=128)
                )
                T4 = spool.tile([128, 4, m], F32, tag="T4", name=f"T4_{nm}")
                nc.vector.tensor_tensor(T4[:], R[:, 0:4, :], R[:, 4:8, :], ADD)
                T2 = spool.tile([128, 2, m], F32, tag="T2", name=f"T2_{nm}")
                nc.vector.tensor_tensor(T2[:], T4[:, 0:2, :], T4[:, 2:4, :], ADD)
                X = spool.tile([128, m], F32, tag="X", name=f"X_{nm}")
                nc.vector.tensor_tensor(X[:], T2[:, 0, :], T2[:, 1, :], ADD)
                if split:
                    # batch-half-split stats: X = [block_h0(2n) | block_h1(2n)]
                    # where each block is [sums(n), sqs(n)]
                    SS = spool.tile([128, 2 * n], F32, tag="SS", name=f"SS_{nm}")
                    nc.vector.tensor_tensor(SS[:], X[:, 0:2*n], X[:, 2*n:4*n], ADD)
                else:
                    SS = X  # layout [sums(n), sqs(n)]
                T = spool.tile([128, 2 * n], F32, tag="T", name=f"T_{nm}")
                nc.vector.tensor_scalar_mul(T[:], SS[:, 0 : 2 * n], 1.0 / B)
                MEAN = T[:, 0:n]
                MSQ = spool.tile([128, n], F32, tag="MSQ", name=f"MSQ_{nm}")
                nc.vector.tensor_tensor(MSQ[:], MEAN, MEAN, MUL)
                VAR = spool.tile([128, n], F32, tag="VAR", name=f"VAR_{nm}")
                # (sq/B + eps) - mean^2  in one op
                nc.vector.scalar_tensor_tensor(
                    VAR[:], T[:, n : 2 * n], EPS, MSQ[:], ADD, SUB
                )
                RINV = spool.tile([128, n], F32, tag="RINV", name=f"RINV_{nm}")
                nc.vector.reciprocal(RINV[:], VAR[:])
                RSTD = spool.tile([128, n], F32, tag="RSTD", name=f"RSTD_{nm}")
                nc.scalar.sqrt(RSTD[:], RINV[:])
                g0 = (2 * li) * JT + jts[0]
                b0 = (2 * li + 1) * JT + jts[0]
                A = spool.tile([128, n], F32, tag="A", name=f"A_{nm}")
                nc.vector.tensor_tensor(A[:], RSTD[:], BNP[:, g0 : g0 + n], MUL)
                AM = spool.tile([128, n], F32, tag="AM", name=f"AM_{nm}")
                nc.vector.tensor_tensor(AM[:], A[:], MEAN, MUL)
                C = spool.tile([128, n], F32, tag="C", name=f"C_{nm}")
                nc.vector.tensor_tensor(C[:], BNP[:, b0 : b0 + n], AM[:], SUB)
                for j, jt in enumerate(jts):
                    # apply in batch-halves: the consumer matmul of this tile
                    # (next layer's k-step / L4) reads halves in order, so the
                    # h0 half-apply unblocks it ~0.25us earlier than a full
                    # -width apply would
                    for s, w in NHALVES:
                        nc.vector.tensor_scalar(
                            Hdst[:, jt, s : s + w],
                            HRAW[:, jt, s : s + w],
                            A[:, j : j + 1],
                            C[:, j : j + 1],
                            MUL,
                            ADD,
                        )

            def mlp_layer(li, kt, rhs, W, Hdst):
                """One layer: matmuls + relu + distributed BN into Hdst.

                Issue-order design (per engine):
                  Tensor: g0 k-outer (4 chains), chains j4, j5, j6, j7(split)
                  Vector: sumsq j0..3, j4, j5, fin(g0), sumsq j6, j7h0, j7h1,
                          fin(g1), fin(g2), fin(g3) — each fin's AllReduce has
                          landed (or nearly) by the time Vector reaches it.
                  GpSimd: cc_in kick + AR trigger per group (no gathers here,
                          so a pending AR never blocks the next group's kick).
                """
                # group 0: k-outer over 4 concurrent psum chains
                S0 = spool.tile([128, 8], F32, tag="S0", name=f"S0_{li}")
                pss = [
                    pspool.tile([128, BL], F32, tag="ps", name=f"ps{li}_g{j}")
                    for j in range(4)
                ]
                for k in range(kt):
                    for j in range(4):
                        mm_pair(pss[j], W[:, k, j * 128 : (j + 1) * 128], rhs, k, kt)
                for j in range(4):
                    relu_sumsq(pss[j], j, S0, j, 4)
                cc0 = ar_start(li, 0, S0)

                # chains j4, j5
                S1 = spool.tile([128, 4], F32, tag="S1", name=f"S1_{li}")
                for i, jt in enumerate((4, 5)):
                    ps = pspool.tile([128, BL], F32, tag="ps", name=f"ps{li}_{jt}")
                    for k in range(kt):
                        mm_pair(ps, W[:, k, jt * 128 : (jt + 1) * 128], rhs, k, kt)
                    relu_sumsq(ps, jt, S1, i, 2)
                cc1 = ar_start(li, 1, S1)

                if li == 0:
                    # finish g0 now: applies t0-3 must land before the next
                    # layer's k=0, and L1's long chains leave AG(g0) landed
                    # by the time Vector reaches this
                    bn_finish(li, 0, cc0, Hdst)

                # chains j6, j7: halves k-major so the group's stats land
                # right after j7's last matmul. Per-half S2 block layout
                # [sum6, sum7, sq6, sq7] so bn_finish can fold halves with
                # one contiguous add.
                S2 = spool.tile([128, 8], F32, tag="S2", name=f"S2_{li}")
                for i, jt in enumerate((6, 7)):
                    ps = pspool.tile([128, BL], F32, tag="ps", name=f"ps{li}_{jt}")
                    for h, (s, w) in enumerate(NHALVES):
                        for k in range(kt):
                            nc.tensor.matmul(
                                ps[:, s : s + w],
                                W[:, k, jt * 128 : (jt + 1) * 128],
                                rhs[:, k, s : s + w],
                                start=(k == 0),
                                stop=(k == kt - 1),
                            )
                        nc.scalar.activation(
                            HRAW[:, jt, s : s + w], ps[:, s : s + w], RELU,
                            accum_out=S2[:, 4 * h + i : 4 * h + i + 1],
                        )
                        scr5 = scrpool.tile([128, 512], BF16, tag="scr5",
                                            name=f"scr5_{li}_{jt}_{h}")
                        nc.vector.scalar_tensor_tensor(
                            scr5[:], HRAW[:, jt, s : s + w], 0.0,
                            HRAW[:, jt, s : s + w], BYP, MUL,
                            accum_out=S2[:, 4 * h + 2 + i : 4 * h + 3 + i],
                        )
                cc2 = ar_start(li, 2, S2)

                if li > 0:
                    # deferred g0 finish: keeps a late AllGather from blocking
                    # the j6/j7 stats on the Vector queue (L2/L3 chains leave
                    # enough runway before the next layer's k=0 deadline)
                    bn_finish(li, 0, cc0, Hdst)
                bn_finish(li, 1, cc1, Hdst)
                bn_finish(li, 2, cc2, Hdst, split=True)

            # ---- layers ----
            mlp_layer(0, KT_IN, XT, W1, H)
            mlp_layer(1, JT, H, W2, H2)
            mlp_layer(2, JT, H2, W3, H)

            # ---- layer 4 (no relu/BN) ----
            ps4 = pspool.tile([CLSP, BL], F32, tag="ps", name="ps4")
            for k in range(JT):
                mm_pair(ps4, W4[:, k, :], H, k, JT)
            # warmup anchor (keeps the warmup collective from DCE); nothing
            # is queued behind it on Sync except the final output DMAs
            nc.sync.dma_start(out_d[CLSP - 1 : CLSP, 0:1], wu_gat[0:1, :])
            OUTS = spool.tile([CLSP, BL], F32, tag="OUTS")
            # copy + ship per batch-half: the half-0 output DMA overlaps the
            # half-1 copy, pulling the kernel's end (the final DMA) earlier
            for s, w in NHALVES:
                nc.scalar.copy(OUTS[:, s : s + w], ps4[:, s : s + w])
                nc.sync.dma_start(
                    out_d[: CLSP - 1, s : s + w], OUTS[: CLSP - 1, s : s + w]
                )

    nc.compile()
    return nc


def _get_nc():
    if "nc" not in _CACHE:
        _CACHE["nc"] = _build()
    return _CACHE["nc"]


def _prep_inputs(x, W1, W2, W3, W4, g1, b1, g2, b2, g3, b3):
    x2 = np.asarray(x, dtype=np.float32).reshape(B, KIN)
    xt = np.ascontiguousarray(x2.T).astype(ml_dtypes.bfloat16)  # [3072, 8192]

    def pmajor(a):
        # [ktiles*128, free] -> [128, ktiles, free] (partition-major)
        kt = a.shape[0] // 128
        return np.ascontiguousarray(
            a.reshape(kt, 128, a.shape[1]).transpose(1, 0, 2)
        )

    def bin_t(w, pad=None):
        wb = np.where(np.asarray(w, dtype=np.float32) >= 0, 1.0, -1.0)
        wt = np.ascontiguousarray(wb.T).astype(ml_dtypes.float8_e4m3)  # [in, out]
        if pad is not None and wt.shape[1] < pad:
            wt = np.concatenate(
                [wt, np.zeros((wt.shape[0], pad - wt.shape[1]), wt.dtype)], axis=1
            )
        return pmajor(wt)

    w1t = bin_t(W1)            # [128, 24, 1024]
    w2t = bin_t(W2)            # [128, 8, 1024]
    w3t = bin_t(W3)
    w4t = bin_t(W4, pad=CLSP)  # [128, 8, 16]

    bnp = np.zeros((128, 6 * JT), dtype=np.float32)
    for l, p in enumerate([g1, b1, g2, b2, g3, b3]):
        pa = np.asarray(p, dtype=np.float32)
        for jt in range(JT):
            bnp[:, l * JT + jt] = pa[jt * 128 : (jt + 1) * 128]

    shared = {"w1t": w1t, "w2t": w2t, "w3t": w3t, "w4t": w4t, "bnp": bnp}
    in_maps = []
    for c in range(NCORES):
        m = dict(shared)
        m["xt"] = pmajor(np.ascontiguousarray(xt[:, c * BL : (c + 1) * BL]))
        in_maps.append(m)
    return in_maps


def _run(inputs, trace=False):
    nc = _get_nc()
    in_maps = _prep_inputs(**inputs)
    res = bass_utils.run_bass_kernel_spmd(
        nc, in_maps, core_ids=list(range(NCORES)), trace=trace
    )
    out = np.empty((B, CLS), dtype=np.float32)
    for c in range(NCORES):
        out[c * BL : (c + 1) * BL, :] = res.results[c]["out"][:CLS, :].T
    return out, res


def kernel(**inputs):
    out, _ = _run(inputs, trace=False)
    return out
